# revision 32
# baseline (speedup 1.0000x reference)
"""Trainium2 Bass kernel for nn_Encoder_ATTENTION (gnn_message_passing).

Math (per (b, n)):
  wn     = normalize(w_r_weight[rid[b,n]])            (host table prep)
  d      = <e[b,n,:], wn>
  e_tr   = e - d * wn                                  (unmasked; mask folded into coeffs)
  h      = tanh(W @ [z_q[b]; e_tr] + bias)             (z-part via host-computed zw stream)
  alpha  = u_a . h + u_a_b
  E      = exp(alpha) * (rid < CNT_E)
  attn   = E / sum_n(E) + rw                           (softmax w/o max-sub; logits are small)
  out[b] = sum_n (attn * mask) * e_tr

Sharding: data-parallel over batch, 512 batch rows per core x 8 cores.
Host does layout/table prep: bf16/fp8 conversion, wn-table gather
expansion (wn[rid] streamed row-major), rw masking, and the tiny
zw = Wz @ zq[q_rid] + bias table (134 MFLOP once on host, streamed as
bf16 pairs). All per-(b,n) math runs on device.

V3 highlights (baseline ~1.0ms, V2 1.49ms):
 - e_tr dot+projection: 2 fused DVE scalar_tensor_tensor ops
 - pre-activation matmul: single fp8e4m3 DoubleRow matmul (contract 256
   in one pass, half cycle per row)
 - e_trT via PE transposes (XBAR DMA transposes were 1.2us each on the
   sync queue in V2 - reverted)
 - alpha (u.h): multiply on Pool engine, reduce on DVE
 - input streams split across the two HWDGE issue queues (e on sync,
   wn on scalar)
 - zw stream: one slab DMA per 8 tiles instead of 256 tiny stage DMAs
"""

import sys

import numpy as np


def _ensure_path():
    for p in ("/opt/trn_rl_repo", "/root/.axon_site/_ro/trn_rl_repo"):
        if p not in sys.path:
            sys.path.append(p)


_ensure_path()

from contextlib import ExitStack

import concourse.bacc as bacc
import concourse.bass as bass
import concourse.tile as tile
from concourse import mybir
from concourse.masks import make_identity

B, NB, DIM = 4096, 64, 256
H = 2 * DIM
NCORES = 8
BC = B // NCORES            # 512 batch rows per core
ROWS = BC * NB              # 32768 (b, n) rows per core
NT = ROWS // 128            # 256 tiles of 128 rows
TPB = 16                    # tiles per batch (softmax/output batching)
NBATCH = NT // TPB          # 32
CNT_E = 1000                # padding relation id
N_WR = CNT_E + 1            # 1001 rows in w_r table

f32 = mybir.dt.float32
f32r = mybir.dt.float32r
bf16 = mybir.dt.bfloat16
fp8 = mybir.dt.float8e4
i32 = mybir.dt.int32
AF = mybir.ActivationFunctionType
OP = mybir.AluOpType
DR = mybir.MatmulPerfMode.DoubleRow


def build_nc(nbatch=NBATCH):
    nc = bacc.Bacc("TRN2")

    e_d = nc.dram_tensor("e", [ROWS, DIM], bf16, kind="ExternalInput")
    wn_d = nc.dram_tensor("wn", [ROWS, DIM], bf16, kind="ExternalInput")
    maskT_d = nc.dram_tensor("maskT", [128, NT], f32, kind="ExternalInput")
    rwmT_d = nc.dram_tensor("rwmT", [128, NT], f32, kind="ExternalInput")
    zwst_d = nc.dram_tensor("zwst", [2, NT, H], bf16, kind="ExternalInput")
    WT8_d = nc.dram_tensor("WT8", [128, 2, H], fp8, kind="ExternalInput")
    ua_d = nc.dram_tensor("ua", [1, H], f32, kind="ExternalInput")
    uab_d = nc.dram_tensor("uab", [1, 1], f32, kind="ExternalInput")
    out_d = nc.dram_tensor("out", [BC, DIM], f32, kind="ExternalOutput")

    with tile.TileContext(nc) as tc, ExitStack() as ctx:
        const = ctx.enter_context(tc.tile_pool(name="const", bufs=1))
        epool = ctx.enter_context(tc.tile_pool(name="epool", bufs=2))
        wpool = ctx.enter_context(tc.tile_pool(name="wpool", bufs=2))
        etrp = ctx.enter_context(tc.tile_pool(name="etrp", bufs=34))
        etp = ctx.enter_context(tc.tile_pool(name="etp", bufs=4))
        scp = ctx.enter_context(tc.tile_pool(name="scp", bufs=4))
        sc2p = ctx.enter_context(tc.tile_pool(name="sc2p", bufs=4))
        dvp = ctx.enter_context(tc.tile_pool(name="dvp", bufs=3))
        hpool = ctx.enter_context(tc.tile_pool(name="hpool", bufs=4))
        abp = ctx.enter_context(tc.tile_pool(name="abp", bufs=3))
        czp = ctx.enter_context(tc.tile_pool(name="czp", bufs=2))
        osp = ctx.enter_context(tc.tile_pool(name="osp", bufs=2))
        stp = ctx.enter_context(tc.tile_pool(name="stp", bufs=3))
        rsp = ctx.enter_context(tc.tile_pool(name="rsp", bufs=2))

        hps = ctx.enter_context(tc.tile_pool(name="hps", bufs=3, space="PSUM"))
        tpps = ctx.enter_context(tc.tile_pool(name="tpps", bufs=3, space="PSUM"))
        ops_ = ctx.enter_context(tc.tile_pool(name="ops", bufs=1, space="PSUM"))
        tailp = ctx.enter_context(tc.tile_pool(name="tailp", bufs=1, space="PSUM"))

        # ---------- constants ----------
        ident_f = const.tile([128, 128], f32)
        make_identity(nc, ident_f[:])
        ident = const.tile([128, 128], bf16)
        nc.vector.tensor_copy(ident[:], ident_f[:])

        # blkpat[p, g] = 1.0 if p // 64 == g else 0.0          [128, 2]
        io2 = const.tile([128, 2], i32)
        nc.gpsimd.iota(io2[:], pattern=[[-64, 2]], base=0, channel_multiplier=1)
        bp0 = const.tile([128, 2], f32)
        bp1 = const.tile([128, 2], f32)
        nc.vector.tensor_scalar(out=bp0[:], in0=io2[:], scalar1=0, scalar2=None, op0=OP.is_ge)
        nc.vector.tensor_scalar(out=bp1[:], in0=io2[:], scalar1=63, scalar2=None, op0=OP.is_le)
        blkpat = const.tile([128, 2], f32)
        nc.vector.tensor_tensor(out=blkpat[:], in0=bp0[:], in1=bp1[:], op=OP.mult)

        # O2T[g, c] = 1.0 if c // 64 == g else 0.0             [2, 128]
        io3 = const.tile([2, 128], i32)
        nc.gpsimd.iota(io3[:], pattern=[[1, 128]], base=0, channel_multiplier=-64)
        ot0 = const.tile([2, 128], f32)
        ot1 = const.tile([2, 128], f32)
        nc.vector.tensor_scalar(out=ot0[:], in0=io3[:], scalar1=0, scalar2=None, op0=OP.is_ge)
        nc.vector.tensor_scalar(out=ot1[:], in0=io3[:], scalar1=63, scalar2=None, op0=OP.is_le)
        O2T = const.tile([2, 128], f32r)
        nc.vector.tensor_tensor(out=O2T[:], in0=ot0[:], in1=ot1[:], op=OP.mult)
        O2Tb = const.tile([2, 128], bf16)
        nc.vector.tensor_copy(O2Tb[:], O2T[:])

        # ---------- broadcast / table loads ----------
        # (partition-step-0 DMA broadcast crashes the exec unit on this
        # runtime; broadcast across partitions via a PE outer product instead)
        ones1 = const.tile([1, 128], f32)
        nc.gpsimd.memset(ones1[:], 1.0)
        ua_row = const.tile([1, H], f32)
        nc.sync.dma_start(out=ua_row[:], in_=ua_d[:])
        uab_row = const.tile([1, 1], f32)
        nc.sync.dma_start(out=uab_row[:], in_=uab_d[:])

        bc_ps = hps.tile([128, H], f32, tag="hps")
        nc.tensor.matmul(out=bc_ps[:], lhsT=ones1[:], rhs=ua_row[:])
        u16 = const.tile([128, H], bf16)
        nc.scalar.copy(u16[:], bc_ps[:])
        bc_ps3 = hps.tile([128, H], f32, tag="hps")
        nc.tensor.matmul(out=bc_ps3[:, 0:1], lhsT=ones1[:], rhs=uab_row[:])
        uab_b = const.tile([128, 1], f32)
        nc.scalar.copy(uab_b[:], bc_ps3[:, 0:1])

        WT8 = const.tile([128, 2, H], fp8)
        nc.sync.dma_start(out=WT8[:], in_=WT8_d[:])
        maskT = const.tile([128, NT], f32)
        nc.sync.dma_start(out=maskT[:], in_=maskT_d[:])
        rwmT = const.tile([128, NT], f32)
        nc.sync.dma_start(out=rwmT[:], in_=rwmT_d[:])

        # ---------- main loop ----------
        e_re = e_d[:].rearrange("(t p) d -> p t d", p=128)   # [128, NT, DIM]
        wn_re = wn_d[:].rearrange("(t p) d -> p t d", p=128)

        for bt in range(nbatch):
            t0 = bt * TPB
            e8 = epool.tile([128, TPB, DIM], bf16, tag="e8")
            nc.sync.dma_start(out=e8[:], in_=e_re[:, t0 : t0 + TPB, :])
            w8 = wpool.tile([128, TPB, DIM], bf16, tag="w8")
            nc.scalar.dma_start(out=w8[:], in_=wn_re[:, t0 : t0 + TPB, :])
            st8 = stp.tile([2, TPB, H], bf16, tag="st8")
            nc.sync.dma_start(out=st8[:], in_=zwst_d[:, t0 : t0 + TPB, :])

            alpha_b = abp.tile([128, TPB], f32, tag="alpha")
            HB = TPB // 2
            # batched dot products: X = e*G and a negated reduce, one op per
            # half-group (DVE fixed overhead amortized 8x vs per-tile stt)
            negd_b = dvp.tile([128, TPB], f32, tag="negd")
            xh = scp.tile([128, HB, DIM], bf16, tag="xh")
            nc.vector.tensor_tensor(out=xh[:], in0=e8[:, 0:HB, :], in1=w8[:, 0:HB, :], op=OP.mult)
            nc.vector.tensor_reduce(
                out=negd_b[:, 0:HB], in_=xh[:], axis=mybir.AxisListType.X,
                op=OP.add, negate=True,
            )
            etrs = []
            scr2q = None
            for s in range(TPB):
                if s == 2:
                    # second half-group dots, emitted after the pipeline fills
                    xh2 = scp.tile([128, HB, DIM], bf16, tag="xh")
                    nc.vector.tensor_tensor(
                        out=xh2[:], in0=e8[:, HB:TPB, :], in1=w8[:, HB:TPB, :], op=OP.mult
                    )
                    nc.vector.tensor_reduce(
                        out=negd_b[:, HB:TPB], in_=xh2[:], axis=mybir.AxisListType.X,
                        op=OP.add, negate=True,
                    )
                et = e8[:, s, :]
                G = w8[:, s, :]
                # etr = (G * negd) + e
                etr = etrp.tile([128, DIM], bf16, tag="etr")
                nc.vector.scalar_tensor_tensor(
                    out=etr[:], in0=G, scalar=negd_b[:, s : s + 1], in1=et,
                    op0=OP.mult, op1=OP.add,
                )
                etrs.append(etr)

                # e_trT via PE transposes -> PSUM(bf16) -> fp8 SBUF copy
                tp = tpps.tile([128, 2, 128], bf16, tag="tp")
                for k in range(2):
                    nc.tensor.transpose(
                        out=tp[:, k, :],
                        in_=etr[:, 128 * k : 128 * (k + 1)],
                        identity=ident[:],
                    )
                eT = etp.tile([128, 2, 128], fp8, tag="eT")
                nc.scalar.copy(eT[:], tp[:])

                # pre = W_e @ e_tr  (one fp8 DoubleRow matmul, contract 256)
                h_ps = hps.tile([128, H], f32, tag="hps")
                nc.tensor.matmul(
                    out=h_ps[:], lhsT=eT[:], rhs=WT8[:],
                    start=True, stop=False, perf_mode=DR, skip_group_check=True,
                )
                # + zw pair (broadcast via O2T outer product)
                nc.tensor.matmul(
                    out=h_ps[:], lhsT=O2Tb[:], rhs=st8[:, s, :],
                    start=False, stop=True, skip_group_check=True,
                )

                h = hpool.tile([128, H], bf16, tag="h")
                nc.scalar.activation(out=h[:], in_=h_ps[:], func=AF.Tanh)
                # alpha: multiply on Pool into 4-tile slabs, one batched DVE
                # reduce per slab (amortizes the reduce's fixed cost 4x)
                if s % 4 == 0:
                    scr2q = sc2p.tile([128, 4, H], bf16, tag="scr2")
                nc.gpsimd.tensor_tensor(out=scr2q[:, s % 4, :], in0=h[:], in1=u16[:], op=OP.mult)
                if s % 4 == 3:
                    nc.vector.tensor_reduce(
                        out=alpha_b[:, s - 3 : s + 1], in_=scr2q[:],
                        axis=mybir.AxisListType.X, op=OP.add,
                    )

            # ----- batch tail: softmax + coeffs + output reduction -----
            Eb = abp.tile([128, TPB], f32, tag="Eb")
            nc.scalar.activation(out=Eb[:], in_=alpha_b[:], func=AF.Exp, bias=uab_b[:, 0:1])
            Em = abp.tile([128, TPB], f32, tag="Em")
            nc.vector.tensor_tensor(out=Em[:], in0=Eb[:], in1=maskT[:, t0 : t0 + TPB], op=OP.mult)

            tail_ps = tailp.tile([128, TPB], f32, tag="tail")
            nc.tensor.matmul(out=tail_ps[0:2, :], lhsT=blkpat[:], rhs=Em[:])
            rS = rsp.tile([2, TPB], f32, tag="rS")
            nc.vector.reciprocal(rS[:], tail_ps[0:2, :])
            rS_r = rsp.tile([2, TPB], f32r, tag="rSr")
            nc.vector.tensor_copy(rS_r[:], rS[:])
            rbc_ps = tailp.tile([128, TPB], f32, tag="tail")
            nc.tensor.matmul(out=rbc_ps[:], lhsT=O2T[:], rhs=rS_r[:])

            coeff = abp.tile([128, TPB], f32, tag="coeff")
            nc.vector.tensor_tensor(out=coeff[:], in0=Em[:], in1=rbc_ps[:], op=OP.mult)
            nc.vector.tensor_tensor(out=coeff[:], in0=coeff[:], in1=rwmT[:, t0 : t0 + TPB], op=OP.add)

            # Cz: [128, TPB*16]; block s has coeff at cols (2s, 2s+1), zeros elsewhere
            W2 = 2 * TPB
            cz = czp.tile([128, TPB * W2], bf16, tag="cz")
            nc.gpsimd.memset(cz[:], 0.0)
            for s in range(TPB):
                nc.gpsimd.tensor_scalar(
                    out=cz[:, W2 * s + 2 * s : W2 * s + 2 * s + 2],
                    in0=blkpat[:],
                    scalar1=coeff[:, s : s + 1],
                    scalar2=None,
                    op0=OP.mult,
                )

            o_ps = ops_.tile([2 * TPB, DIM], f32, tag="ops")
            for s in range(TPB):
                nc.tensor.matmul(
                    out=o_ps[:],
                    lhsT=cz[:, W2 * s : W2 * (s + 1)],
                    rhs=etrs[s][:],
                    start=(s == 0),
                    stop=(s == TPB - 1),
                    skip_group_check=True,
                )
            outS = osp.tile([2 * TPB, DIM], f32, tag="outS")
            nc.scalar.copy(outS[:], o_ps[:])
            nc.sync.dma_start(out=out_d[2 * TPB * bt : 2 * TPB * (bt + 1), :], in_=outS[:])

    nc.finalize()
    return nc


_NC = None


def _get_nc():
    global _NC
    if _NC is None:
        _NC = build_nc()
    return _NC


def _prep_in_maps(inputs):
    np_bf16 = mybir.dt.np(bf16)
    np_fp8 = mybir.dt.np(fp8)

    e = np.asarray(inputs["batch_nei_e_emb"], dtype=np.float32)
    rid = np.asarray(inputs["batch_nei_rid"]).astype(np.int32)
    rw = np.asarray(inputs["batch_nei_rw"], dtype=np.float32)
    qr = np.asarray(inputs["batch_q_rid"]).astype(np.int32)

    w = np.asarray(inputs["w_r_weight"], dtype=np.float32)
    nrm = np.maximum(np.linalg.norm(w, axis=1, keepdims=True), 1e-12)
    wn_tab = (w / nrm).astype(np_bf16)                       # [N_WR, DIM]
    mask = (rid < CNT_E).astype(np.float32)                  # [B, NB]
    rwm = rw * mask

    W = np.asarray(inputs["attn_W_w"], dtype=np.float32)     # [out=512, in=512]
    bias = np.asarray(inputs["attn_W_b"], dtype=np.float32)
    # e-part of W, transposed, k-chunk pairs: [128, 2, H] fp8
    WTe = W[:, DIM:].T                                       # [256, 512]
    WT8 = np.ascontiguousarray(WTe.reshape(2, 128, H).transpose(1, 0, 2)).astype(np_fp8)
    # zw[b] = W_z @ zq[q_rid[b]] + bias  (host GEMM, tiny)
    zq = np.asarray(inputs["zq_weight"], dtype=np.float32)
    zw_full = zq[qr] @ W[:, :DIM].T + bias                   # [B, H]
    ua = np.asarray(inputs["u_a_w"], dtype=np.float32).reshape(1, H)
    uab = np.asarray(inputs["u_a_b"], dtype=np.float32).reshape(1, 1)

    e16 = e.astype(np_bf16)

    in_maps = []
    for c in range(NCORES):
        sl = slice(BC * c, BC * (c + 1))
        ec = np.ascontiguousarray(e16[sl].reshape(ROWS, DIM))
        ridc = rid[sl].reshape(ROWS)
        wnc = np.ascontiguousarray(wn_tab[ridc])             # [ROWS, DIM] bf16
        maskc = mask[sl].reshape(ROWS)
        rwmc = rwm[sl].reshape(ROWS)
        # zw pairs: zwst[i, t, :] = zw[BC*c + 2t + i]
        zwc = zw_full[sl].reshape(NT, 2, H).transpose(1, 0, 2)
        in_maps.append(
            {
                "e": ec,
                "wn": wnc,
                "maskT": np.ascontiguousarray(maskc.reshape(NT, 128).T),
                "rwmT": np.ascontiguousarray(rwmc.reshape(NT, 128).T),
                "zwst": np.ascontiguousarray(zwc).astype(np_bf16),
                "WT8": WT8,
                "ua": ua,
                "uab": uab,
            }
        )
    return in_maps


def run_cores(inputs, trace=False, tmpdir=None):
    from concourse.bass_utils import run_bass_kernel_spmd

    nc = _get_nc()
    in_maps = _prep_in_maps(inputs)
    res = run_bass_kernel_spmd(
        nc, in_maps, core_ids=list(range(NCORES)), trace=trace, tmpdir=tmpdir
    )
    out = np.concatenate([res.results[c]["out"] for c in range(NCORES)], axis=0)
    return out, res


def kernel(**inputs):
    out, _ = run_cores(inputs, trace=False)
    return out


# revision 33
# speedup vs baseline: 1.0231x; 1.0231x over previous
"""Trainium2 Bass kernel for nn_Encoder_ATTENTION (gnn_message_passing).

Math (per (b, n)):
  wn     = normalize(w_r_weight[rid[b,n]])            (host table prep)
  d      = <e[b,n,:], wn>
  e_tr   = e - d * wn                                  (unmasked; mask folded into coeffs)
  h      = tanh(W @ [z_q[b]; e_tr] + bias)             (z-part via host-computed zw stream)
  alpha  = u_a . h + u_a_b
  E      = exp(alpha) * (rid < CNT_E)
  attn   = E / sum_n(E) + rw                           (softmax w/o max-sub; logits are small)
  out[b] = sum_n (attn * mask) * e_tr

Sharding: data-parallel over batch, 512 batch rows per core x 8 cores.
Host does layout/table prep: bf16/fp8 conversion, wn-table gather
expansion (wn[rid] streamed row-major), rw masking, and the tiny
zw = Wz @ zq[q_rid] + bias table (134 MFLOP once on host, streamed as
bf16 pairs). All per-(b,n) math runs on device.

V3 highlights (baseline ~1.0ms, V2 1.49ms):
 - e_tr dot+projection: 2 fused DVE scalar_tensor_tensor ops
 - pre-activation matmul: single fp8e4m3 DoubleRow matmul (contract 256
   in one pass, half cycle per row)
 - e_trT via PE transposes (XBAR DMA transposes were 1.2us each on the
   sync queue in V2 - reverted)
 - alpha (u.h): multiply on Pool engine, reduce on DVE
 - input streams split across the two HWDGE issue queues (e on sync,
   wn on scalar)
 - zw stream: one slab DMA per 8 tiles instead of 256 tiny stage DMAs
"""

import sys

import numpy as np


def _ensure_path():
    for p in ("/opt/trn_rl_repo", "/root/.axon_site/_ro/trn_rl_repo"):
        if p not in sys.path:
            sys.path.append(p)


_ensure_path()

from contextlib import ExitStack

import concourse.bacc as bacc
import concourse.bass as bass
import concourse.tile as tile
from concourse import mybir
from concourse.masks import make_identity

B, NB, DIM = 4096, 64, 256
H = 2 * DIM
NCORES = 8
BC = B // NCORES            # 512 batch rows per core
ROWS = BC * NB              # 32768 (b, n) rows per core
NT = ROWS // 128            # 256 tiles of 128 rows
TPB = 16                    # tiles per batch (softmax/output batching)
NBATCH = NT // TPB          # 32
CNT_E = 1000                # padding relation id
N_WR = CNT_E + 1            # 1001 rows in w_r table

f32 = mybir.dt.float32
f32r = mybir.dt.float32r
bf16 = mybir.dt.bfloat16
fp8 = mybir.dt.float8e4
i32 = mybir.dt.int32
AF = mybir.ActivationFunctionType
OP = mybir.AluOpType
DR = mybir.MatmulPerfMode.DoubleRow


def build_nc(nbatch=NBATCH):
    nc = bacc.Bacc("TRN2")

    e_d = nc.dram_tensor("e", [ROWS, DIM], bf16, kind="ExternalInput")
    wn_d = nc.dram_tensor("wn", [ROWS, DIM], bf16, kind="ExternalInput")
    maskT_d = nc.dram_tensor("maskT", [128, NT], f32, kind="ExternalInput")
    rwmT_d = nc.dram_tensor("rwmT", [128, NT], f32, kind="ExternalInput")
    zwst_d = nc.dram_tensor("zwst", [2, NT, H], bf16, kind="ExternalInput")
    WT8_d = nc.dram_tensor("WT8", [128, 2, H], fp8, kind="ExternalInput")
    ua_d = nc.dram_tensor("ua", [1, H], f32, kind="ExternalInput")
    uab_d = nc.dram_tensor("uab", [1, 1], f32, kind="ExternalInput")
    out_d = nc.dram_tensor("out", [BC, DIM], f32, kind="ExternalOutput")

    with tile.TileContext(nc) as tc, ExitStack() as ctx:
        const = ctx.enter_context(tc.tile_pool(name="const", bufs=1))
        epool = ctx.enter_context(tc.tile_pool(name="epool", bufs=2))
        wpool = ctx.enter_context(tc.tile_pool(name="wpool", bufs=2))
        etrp = ctx.enter_context(tc.tile_pool(name="etrp", bufs=34))
        etp = ctx.enter_context(tc.tile_pool(name="etp", bufs=4))
        scp = ctx.enter_context(tc.tile_pool(name="scp", bufs=4))
        sc2p = ctx.enter_context(tc.tile_pool(name="sc2p", bufs=4))
        dvp = ctx.enter_context(tc.tile_pool(name="dvp", bufs=3))
        hpool = ctx.enter_context(tc.tile_pool(name="hpool", bufs=4))
        abp = ctx.enter_context(tc.tile_pool(name="abp", bufs=3))
        czp = ctx.enter_context(tc.tile_pool(name="czp", bufs=2))
        osp = ctx.enter_context(tc.tile_pool(name="osp", bufs=2))
        stp = ctx.enter_context(tc.tile_pool(name="stp", bufs=3))
        rsp = ctx.enter_context(tc.tile_pool(name="rsp", bufs=2))

        hps = ctx.enter_context(tc.tile_pool(name="hps", bufs=3, space="PSUM"))
        tpps = ctx.enter_context(tc.tile_pool(name="tpps", bufs=2, space="PSUM"))
        ops_ = ctx.enter_context(tc.tile_pool(name="ops", bufs=1, space="PSUM"))
        sps = ctx.enter_context(tc.tile_pool(name="sps", bufs=1, space="PSUM"))
        rbcp = ctx.enter_context(tc.tile_pool(name="rbcp", bufs=1, space="PSUM"))

        # ---------- constants ----------
        ident_f = const.tile([128, 128], f32)
        make_identity(nc, ident_f[:])
        ident = const.tile([128, 128], bf16)
        nc.vector.tensor_copy(ident[:], ident_f[:])

        # blkpat[p, g] = 1.0 if p // 64 == g else 0.0          [128, 2]
        io2 = const.tile([128, 2], i32)
        nc.gpsimd.iota(io2[:], pattern=[[-64, 2]], base=0, channel_multiplier=1)
        bp0 = const.tile([128, 2], f32)
        bp1 = const.tile([128, 2], f32)
        nc.vector.tensor_scalar(out=bp0[:], in0=io2[:], scalar1=0, scalar2=None, op0=OP.is_ge)
        nc.vector.tensor_scalar(out=bp1[:], in0=io2[:], scalar1=63, scalar2=None, op0=OP.is_le)
        blkpat = const.tile([128, 2], f32)
        nc.vector.tensor_tensor(out=blkpat[:], in0=bp0[:], in1=bp1[:], op=OP.mult)

        # O2T[g, c] = 1.0 if c // 64 == g else 0.0             [2, 128]
        io3 = const.tile([2, 128], i32)
        nc.gpsimd.iota(io3[:], pattern=[[1, 128]], base=0, channel_multiplier=-64)
        ot0 = const.tile([2, 128], f32)
        ot1 = const.tile([2, 128], f32)
        nc.vector.tensor_scalar(out=ot0[:], in0=io3[:], scalar1=0, scalar2=None, op0=OP.is_ge)
        nc.vector.tensor_scalar(out=ot1[:], in0=io3[:], scalar1=63, scalar2=None, op0=OP.is_le)
        O2T = const.tile([2, 128], f32r)
        nc.vector.tensor_tensor(out=O2T[:], in0=ot0[:], in1=ot1[:], op=OP.mult)
        O2Tb = const.tile([2, 128], bf16)
        nc.vector.tensor_copy(O2Tb[:], O2T[:])

        # ---------- broadcast / table loads ----------
        # (partition-step-0 DMA broadcast crashes the exec unit on this
        # runtime; broadcast across partitions via a PE outer product instead)
        ones1 = const.tile([1, 128], f32)
        nc.gpsimd.memset(ones1[:], 1.0)
        ua_row = const.tile([1, H], f32)
        nc.sync.dma_start(out=ua_row[:], in_=ua_d[:])
        uab_row = const.tile([1, 1], f32)
        nc.sync.dma_start(out=uab_row[:], in_=uab_d[:])

        bc_ps = hps.tile([128, H], f32, tag="hps")
        nc.tensor.matmul(out=bc_ps[:], lhsT=ones1[:], rhs=ua_row[:])
        u16 = const.tile([128, H], bf16)
        nc.scalar.copy(u16[:], bc_ps[:])
        bc_ps3 = hps.tile([128, H], f32, tag="hps")
        nc.tensor.matmul(out=bc_ps3[:, 0:1], lhsT=ones1[:], rhs=uab_row[:])
        uab_b = const.tile([128, 1], f32)
        nc.scalar.copy(uab_b[:], bc_ps3[:, 0:1])

        WT8 = const.tile([128, 2, H], fp8)
        nc.sync.dma_start(out=WT8[:], in_=WT8_d[:])
        maskT = const.tile([128, NT], f32)
        nc.sync.dma_start(out=maskT[:], in_=maskT_d[:])
        rwmT = const.tile([128, NT], f32)
        nc.sync.dma_start(out=rwmT[:], in_=rwmT_d[:])

        # ---------- main loop ----------
        e_re = e_d[:].rearrange("(t p) d -> p t d", p=128)   # [128, NT, DIM]
        wn_re = wn_d[:].rearrange("(t p) d -> p t d", p=128)

        for bt in range(nbatch):
            t0 = bt * TPB
            e8 = epool.tile([128, TPB, DIM], bf16, tag="e8")
            nc.sync.dma_start(out=e8[:], in_=e_re[:, t0 : t0 + TPB, :])
            w8 = wpool.tile([128, TPB, DIM], bf16, tag="w8")
            nc.scalar.dma_start(out=w8[:], in_=wn_re[:, t0 : t0 + TPB, :])
            st8 = stp.tile([2, TPB, H], bf16, tag="st8")
            nc.sync.dma_start(out=st8[:], in_=zwst_d[:, t0 : t0 + TPB, :])

            alpha_b = abp.tile([128, TPB], f32, tag="alpha")
            HB = TPB // 2
            # batched dot products: X = e*G and a negated reduce, one op per
            # half-group (DVE fixed overhead amortized 8x vs per-tile stt)
            negd_b = dvp.tile([128, TPB], f32, tag="negd")
            xh = scp.tile([128, HB, DIM], bf16, tag="xh")
            nc.vector.tensor_tensor(out=xh[:], in0=e8[:, 0:HB, :], in1=w8[:, 0:HB, :], op=OP.mult)
            nc.vector.tensor_reduce(
                out=negd_b[:, 0:HB], in_=xh[:], axis=mybir.AxisListType.X,
                op=OP.add, negate=True,
            )
            etrs = []
            scr2q = None
            for s in range(TPB):
                if s == 2:
                    # second half-group dots, emitted after the pipeline fills
                    xh2 = scp.tile([128, HB, DIM], bf16, tag="xh")
                    nc.vector.tensor_tensor(
                        out=xh2[:], in0=e8[:, HB:TPB, :], in1=w8[:, HB:TPB, :], op=OP.mult
                    )
                    nc.vector.tensor_reduce(
                        out=negd_b[:, HB:TPB], in_=xh2[:], axis=mybir.AxisListType.X,
                        op=OP.add, negate=True,
                    )
                et = e8[:, s, :]
                G = w8[:, s, :]
                # etr = (G * negd) + e
                etr = etrp.tile([128, DIM], bf16, tag="etr")
                nc.vector.scalar_tensor_tensor(
                    out=etr[:], in0=G, scalar=negd_b[:, s : s + 1], in1=et,
                    op0=OP.mult, op1=OP.add,
                )
                etrs.append(etr)

                # e_trT via PE transposes -> PSUM(bf16) -> fp8 SBUF copy
                tp = tpps.tile([128, 2, 128], bf16, tag="tp")
                for k in range(2):
                    nc.tensor.transpose(
                        out=tp[:, k, :],
                        in_=etr[:, 128 * k : 128 * (k + 1)],
                        identity=ident[:],
                    )
                eT = etp.tile([128, 2, 128], fp8, tag="eT")
                nc.scalar.copy(eT[:], tp[:])

                # pre = W_e @ e_tr  (one fp8 DoubleRow matmul, contract 256)
                h_ps = hps.tile([128, H], f32, tag="hps")
                nc.tensor.matmul(
                    out=h_ps[:], lhsT=eT[:], rhs=WT8[:],
                    start=True, stop=False, perf_mode=DR, skip_group_check=True,
                )
                # + zw pair (broadcast via O2T outer product)
                nc.tensor.matmul(
                    out=h_ps[:], lhsT=O2Tb[:], rhs=st8[:, s, :],
                    start=False, stop=True, skip_group_check=True,
                )

                h = hpool.tile([128, H], bf16, tag="h")
                nc.scalar.activation(out=h[:], in_=h_ps[:], func=AF.Tanh)
                # alpha: multiply on Pool into 4-tile slabs, one batched DVE
                # reduce per slab (amortizes the reduce's fixed cost 4x)
                if s % 4 == 0:
                    scr2q = sc2p.tile([128, 4, H], bf16, tag="scr2")
                nc.gpsimd.tensor_tensor(out=scr2q[:, s % 4, :], in0=h[:], in1=u16[:], op=OP.mult)
                if s % 4 == 3:
                    nc.vector.tensor_reduce(
                        out=alpha_b[:, s - 3 : s + 1], in_=scr2q[:],
                        axis=mybir.AxisListType.X, op=OP.add,
                    )

            # ----- batch tail: softmax + coeffs + output reduction -----
            Eb = abp.tile([128, TPB], f32, tag="Eb")
            nc.scalar.activation(out=Eb[:], in_=alpha_b[:], func=AF.Exp, bias=uab_b[:, 0:1])
            Em = abp.tile([128, TPB], f32, tag="Em")
            nc.vector.tensor_tensor(out=Em[:], in0=Eb[:], in1=maskT[:, t0 : t0 + TPB], op=OP.mult)

            s_ps = sps.tile([2, TPB], f32, tag="sps")
            nc.tensor.matmul(out=s_ps[:], lhsT=blkpat[:], rhs=Em[:])
            rS = rsp.tile([2, TPB], f32, tag="rS")
            nc.vector.reciprocal(rS[:], s_ps[:])
            rS_r = rsp.tile([2, TPB], f32r, tag="rSr")
            nc.vector.tensor_copy(rS_r[:], rS[:])
            rbc_ps = rbcp.tile([128, TPB], f32, tag="rbc")
            nc.tensor.matmul(out=rbc_ps[:], lhsT=O2T[:], rhs=rS_r[:])

            coeff = abp.tile([128, TPB], f32, tag="coeff")
            nc.vector.tensor_tensor(out=coeff[:], in0=Em[:], in1=rbc_ps[:], op=OP.mult)
            nc.vector.tensor_tensor(out=coeff[:], in0=coeff[:], in1=rwmT[:, t0 : t0 + TPB], op=OP.add)

            # Cz: [128, TPB*16]; block s has coeff at cols (2s, 2s+1), zeros elsewhere
            W2 = 2 * TPB
            cz = czp.tile([128, TPB * W2], bf16, tag="cz")
            nc.gpsimd.memset(cz[:], 0.0)
            for s in range(TPB):
                nc.gpsimd.tensor_scalar(
                    out=cz[:, W2 * s + 2 * s : W2 * s + 2 * s + 2],
                    in0=blkpat[:],
                    scalar1=coeff[:, s : s + 1],
                    scalar2=None,
                    op0=OP.mult,
                )

            o_ps = ops_.tile([2 * TPB, DIM], f32, tag="ops")
            for s in range(TPB):
                nc.tensor.matmul(
                    out=o_ps[:],
                    lhsT=cz[:, W2 * s : W2 * (s + 1)],
                    rhs=etrs[s][:],
                    start=(s == 0),
                    stop=(s == TPB - 1),
                    skip_group_check=True,
                )
            outS = osp.tile([2 * TPB, DIM], f32, tag="outS")
            nc.scalar.copy(outS[:], o_ps[:])
            nc.sync.dma_start(out=out_d[2 * TPB * bt : 2 * TPB * (bt + 1), :], in_=outS[:])

    nc.finalize()
    return nc


_NC = None


def _get_nc():
    global _NC
    if _NC is None:
        _NC = build_nc()
    return _NC


def _prep_in_maps(inputs):
    np_bf16 = mybir.dt.np(bf16)
    np_fp8 = mybir.dt.np(fp8)

    e = np.asarray(inputs["batch_nei_e_emb"], dtype=np.float32)
    rid = np.asarray(inputs["batch_nei_rid"]).astype(np.int32)
    rw = np.asarray(inputs["batch_nei_rw"], dtype=np.float32)
    qr = np.asarray(inputs["batch_q_rid"]).astype(np.int32)

    w = np.asarray(inputs["w_r_weight"], dtype=np.float32)
    nrm = np.maximum(np.linalg.norm(w, axis=1, keepdims=True), 1e-12)
    wn_tab = (w / nrm).astype(np_bf16)                       # [N_WR, DIM]
    mask = (rid < CNT_E).astype(np.float32)                  # [B, NB]
    rwm = rw * mask

    W = np.asarray(inputs["attn_W_w"], dtype=np.float32)     # [out=512, in=512]
    bias = np.asarray(inputs["attn_W_b"], dtype=np.float32)
    # e-part of W, transposed, k-chunk pairs: [128, 2, H] fp8
    WTe = W[:, DIM:].T                                       # [256, 512]
    WT8 = np.ascontiguousarray(WTe.reshape(2, 128, H).transpose(1, 0, 2)).astype(np_fp8)
    # zw[b] = W_z @ zq[q_rid[b]] + bias  (host GEMM, tiny)
    zq = np.asarray(inputs["zq_weight"], dtype=np.float32)
    zw_full = zq[qr] @ W[:, :DIM].T + bias                   # [B, H]
    ua = np.asarray(inputs["u_a_w"], dtype=np.float32).reshape(1, H)
    uab = np.asarray(inputs["u_a_b"], dtype=np.float32).reshape(1, 1)

    e16 = e.astype(np_bf16)

    in_maps = []
    for c in range(NCORES):
        sl = slice(BC * c, BC * (c + 1))
        ec = np.ascontiguousarray(e16[sl].reshape(ROWS, DIM))
        ridc = rid[sl].reshape(ROWS)
        wnc = np.ascontiguousarray(wn_tab[ridc])             # [ROWS, DIM] bf16
        maskc = mask[sl].reshape(ROWS)
        rwmc = rwm[sl].reshape(ROWS)
        # zw pairs: zwst[i, t, :] = zw[BC*c + 2t + i]
        zwc = zw_full[sl].reshape(NT, 2, H).transpose(1, 0, 2)
        in_maps.append(
            {
                "e": ec,
                "wn": wnc,
                "maskT": np.ascontiguousarray(maskc.reshape(NT, 128).T),
                "rwmT": np.ascontiguousarray(rwmc.reshape(NT, 128).T),
                "zwst": np.ascontiguousarray(zwc).astype(np_bf16),
                "WT8": WT8,
                "ua": ua,
                "uab": uab,
            }
        )
    return in_maps


def run_cores(inputs, trace=False, tmpdir=None):
    from concourse.bass_utils import run_bass_kernel_spmd

    nc = _get_nc()
    in_maps = _prep_in_maps(inputs)
    res = run_bass_kernel_spmd(
        nc, in_maps, core_ids=list(range(NCORES)), trace=trace, tmpdir=tmpdir
    )
    out = np.concatenate([res.results[c]["out"] for c in range(NCORES)], axis=0)
    return out, res


def kernel(**inputs):
    out, _ = run_cores(inputs, trace=False)
    return out
